# revision 14
# baseline (speedup 1.0000x reference)
"""CrossAttentionHead TRN2 kernel.

Full inputs -> full output. Shards batch (B=8) across 8 NeuronCores,
one batch element per core (pure data parallel, no collectives).

Layout choice: each core's x shard is staged host-side as xT = x.T
([E, S], part of sharding prep), so the kernel streams it straight into
the e-on-partitions layout every matmul needs -- no on-chip transpose
pass over x.

Per-core algorithm (xT: [E=768, S=2048], W*: [E, H=128]):
  qT   = Wq.T @ xT + bq                    ([H, S], weights stationary)
  kT   = Wk.T @ xT + bk
  vT   = Wv.T @ xT + bv  -> vN = transpose(vT)   ([S, H] natural)
  for each sq block (512 wide):
    for each sk tile pair (2x128):
      sT   = kT_tile.T @ qT_block          (scores TRANSPOSED [sk, sq])
      es   = exp(sT / sqrt(E))             (ScalarE, scale fused, 1024 wide)
      acc += es                            (DVE, for row sums)
      oT  += vN_tile.T @ es                (PV accumulate, [H, sq])
    rowsum = ones.T @ acc                  ([1, sq] via PE, ones stationary)
    rsT    = transpose(rowsum)             (PE, [sq,1] tiles)
    out    = transpose(oT) * (1/rsT)       -> DMA

Matmul inputs use float32r (fp32 bits streamed through the PE in one
pass, ~2 cyc/row measured, vs plain fp32's 2 half-speed passes at
4 cyc/row; ~1.5e-4 relative rounding per matmul).
Softmax skips max-subtraction: energy/sqrt(768) ~ N(0, 0.41^2) so exp
is safely in range; matches jax.nn.softmax to fp32 rounding.
"""

import sys

if '/opt/trn_rl_repo' not in sys.path:
    sys.path.insert(0, '/opt/trn_rl_repo')

import numpy as np

B, S, E, H = 8, 2048, 768, 128
NCORES = 8
ST = S // 128          # 16 sequence tiles
EC = E // 128          # 6 embed chunks
QB = 4                 # sq blocks
QW = S // QB           # 512 sq block width
SCALE = float(1.0 / np.sqrt(np.float32(E)))

_CACHE = {}
F32R = True


def _build(f32r=F32R):
    import concourse.bacc as bacc
    import concourse.mybir as mybir
    import concourse.tile as tile
    from concourse.masks import make_identity

    dt = mybir.dt
    f32 = dt.float32
    fmm = dt.float32r if f32r else dt.float32
    AF = mybir.ActivationFunctionType

    nc = bacc.Bacc(None, target_bir_lowering=False)
    xT_d = nc.dram_tensor("xT", [E, S], f32, kind="ExternalInput")
    w_d = {}
    b_d = {}
    for nm in ("q", "k", "v"):
        w_d[nm] = nc.dram_tensor(f"W{nm}", [E, H], f32, kind="ExternalInput")
        b_d[nm] = nc.dram_tensor(f"b{nm}", [H], f32, kind="ExternalInput")
    out_d = nc.dram_tensor("out", [S, H], f32, kind="ExternalOutput")

    with tile.TileContext(nc) as tc:
        with tc.tile_pool(name="const", bufs=1) as constp, \
             tc.tile_pool(name="big", bufs=1) as bigp:
            ident = constp.tile([128, 128], f32)
            make_identity(nc, ident[:])
            ones = constp.tile([128, 1], f32)
            nc.vector.memset(ones[:], 1.0)

            # HAM warm-up: dense junk matmuls flip the PE clock gate to
            # 8/8 (~3.4us of activity) while the input DMAs stream in.
            with tc.tile_pool(name="warm_ps", bufs=1, space="PSUM") as wmp:
                wps = wmp.tile([128, 128], f32, tag="warm")
                for _ in range(10):
                    nc.tensor.matmul(wps[:], ident[:], ident[:],
                                     start=True, stop=True)
                wsb = constp.tile([128, 128], f32, name="warm_sink")
                nc.vector.tensor_copy(wsb[:], wps[:])

            w_mm = {}
            b_sb = {}
            for nm in ("q", "k", "v"):
                w_mm[nm] = constp.tile([128, EC, H], fmm, name=f"w_{nm}")
                nc.sync.dma_start(
                    out=w_mm[nm][:],
                    in_=w_d[nm].rearrange("(c p) d -> p c d", p=128).bitcast(fmm))

            xT = []
            for c in range(EC):
                t = bigp.tile([128, S], fmm, name=f"xT{c}")
                for n in range(4):
                    nc.sync.dma_start(
                        out=t[:, n * 512:(n + 1) * 512],
                        in_=xT_d[c * 128:(c + 1) * 128,
                                 n * 512:(n + 1) * 512].bitcast(fmm))
                xT.append(t)

            for nm in ("q", "k", "v"):
                b_sb[nm] = constp.tile([128, 1], f32, name=f"b_{nm}")
                nc.sync.dma_start(out=b_sb[nm][:], in_=b_d[nm][:, None])

            # Projections, split per 512-wide n block: qT/kT/vT = W.T@xT + b
            qT = [bigp.tile([128, QW], fmm, name=f"qT{n}") for n in range(4)]
            kT = [bigp.tile([128, QW], fmm, name=f"kT{n}") for n in range(4)]
            vT = [bigp.tile([128, QW], f32, name=f"vT{n}") for n in range(4)]
            # q/k first with chunk-outer accumulation: every psum tile
            # advances as each xT chunk's DMA lands (no stall on chunk 5)
            with tc.tile_pool(name="proj_ps", bufs=1, space="PSUM") as projp:
                ps_qk = {(nm, n): projp.tile([128, QW], f32,
                                             name=f"ps_{nm}{n}", tag=f"p{nm}{n}")
                         for nm in ("q", "k") for n in range(4)}
                for c in range(EC):
                    for nm in ("q", "k"):
                        for n in range(4):
                            nc.tensor.matmul(
                                ps_qk[(nm, n)][:], w_mm[nm][:, c, :],
                                xT[c][:, n * 512:(n + 1) * 512],
                                start=(c == 0), stop=(c == EC - 1))
                for nm, dst in (("q", qT), ("k", kT)):
                    for n in range(4):
                        nc.scalar.activation(
                            dst[n][:], ps_qk[(nm, n)][:], AF.Identity,
                            bias=b_sb[nm][:], scale=1.0)
                for n in range(4):
                    ps = projp.tile([128, QW], f32, name=f"ps_v{n}",
                                    tag=f"pq{n}")
                    for c in range(EC):
                        nc.tensor.matmul(
                            ps[:], w_mm["v"][:, c, :],
                            xT[c][:, n * 512:(n + 1) * 512],
                            start=(c == 0), stop=(c == EC - 1))
                    nc.scalar.activation(
                        vT[n][:], ps[:], AF.Identity,
                        bias=b_sb["v"][:], scale=1.0)

            # v natural [S, H], one tile per sk tile
            vN = [bigp.tile([128, H], fmm, name=f"vN{t}") for t in range(ST)]
            with tc.tile_pool(name="vt_ps", bufs=4, space="PSUM") as vtp:
                for t in range(ST):
                    pt = vtp.tile([128, 128], f32, tag="vt")
                    nc.tensor.transpose(
                        pt[:], vT[t // 4][:, (t % 4) * 128:(t % 4 + 1) * 128],
                        ident[:])
                    nc.vector.tensor_copy(vN[t][:], pt[:])

            # Main attention loop; kt pairs share one 1024-wide psum tile
            # so exp runs at 1024 elems/op
            with tc.tile_pool(name="s_ps", bufs=2, space="PSUM") as sp, \
                 tc.tile_pool(name="o_ps", bufs=2, space="PSUM") as op, \
                 tc.tile_pool(name="f_ps", bufs=2, space="PSUM") as fp, \
                 tc.tile_pool(name="es_sb", bufs=4) as esp, \
                 tc.tile_pool(name="acc_sb", bufs=2) as accp, \
                 tc.tile_pool(name="o_sb", bufs=2) as osp, \
                 tc.tile_pool(name="small", bufs=4) as smp, \
                 tc.tile_pool(name="fin", bufs=4) as finp:
                for qb in range(QB):
                    oT_ps = op.tile([128, QW], f32, tag="opv")
                    acc2 = accp.tile([128, 2 * QW], f32, tag="acc")
                    for kp in range(ST // 2):
                        kt0, kt1 = 2 * kp, 2 * kp + 1
                        s_ps = sp.tile([128, 2 * QW], f32, tag="s")
                        for i, kt in ((0, kt0), (1, kt1)):
                            nc.tensor.matmul(
                                s_ps[:, i * QW:(i + 1) * QW],
                                kT[kt // 4][:, (kt % 4) * 128:(kt % 4 + 1) * 128],
                                qT[qb][:], start=True, stop=True)
                        es = esp.tile([128, 2 * QW], fmm, tag="es")
                        nc.scalar.activation(es[:], s_ps[:], AF.Exp,
                                             scale=SCALE)
                        if kp == 0:
                            nc.vector.tensor_copy(acc2[:], es[:])
                        else:
                            nc.vector.tensor_add(acc2[:], acc2[:], es[:])
                        for i, kt in ((0, kt0), (1, kt1)):
                            nc.tensor.matmul(
                                oT_ps[:], vN[kt][:], es[:, i * QW:(i + 1) * QW],
                                start=(kt == 0), stop=(kt == ST - 1))
                    # row sums: ones stationary (1-column weight load),
                    # both acc halves accumulate into one [1, 512] bank
                    rs_ps = fp.tile([1, QW], f32, tag="fin")
                    nc.tensor.matmul(rs_ps[:], ones[:], acc2[:, :QW],
                                     start=True, stop=False)
                    nc.tensor.matmul(rs_ps[:], ones[:], acc2[:, QW:],
                                     start=False, stop=True)
                    rs_row = smp.tile([1, QW], f32, tag="rsrow")
                    nc.vector.tensor_copy(rs_row[:], rs_ps[:])
                    oT_sb = osp.tile([128, QW], f32, tag="ot")
                    nc.vector.tensor_copy(oT_sb[:], oT_ps[:])
                    for st in range(4):
                        rsT_ps = fp.tile([128, 1], f32, tag="fin")
                        nc.tensor.transpose(
                            rsT_ps[:], rs_row[:, st * 128:(st + 1) * 128],
                            ident[:1, :1])
                        rcpT = smp.tile([128, 1], f32, tag="rcp")
                        nc.vector.reciprocal(rcpT[:], rsT_ps[:])
                        ot_ps = fp.tile([128, 128], f32, tag="fin")
                        nc.tensor.transpose(
                            ot_ps[:], oT_sb[:, st * 128:(st + 1) * 128],
                            ident[:])
                        o_sb = finp.tile([128, 128], f32, tag="osb")
                        nc.vector.tensor_scalar_mul(o_sb[:], ot_ps[:], rcpT[:])
                        r0 = (qb * 4 + st) * 128
                        nc.sync.dma_start(
                            out=out_d[r0:r0 + 128, :], in_=o_sb[:])

    nc.finalize()
    return nc


def _get_nc():
    if "nc" not in _CACHE:
        _CACHE["nc"] = _build()
    return _CACHE["nc"]


def kernel(x, enc_output, Wq, bq, Wk, bk, Wv, bv):
    from concourse.bass_utils import run_bass_kernel_spmd

    nc = _get_nc()
    x = np.asarray(x, dtype=np.float32)
    in_maps = []
    for b in range(NCORES):
        in_maps.append({
            "xT": np.ascontiguousarray(x[b].T),
            "Wq": np.asarray(Wq, np.float32),
            "bq": np.asarray(bq, np.float32),
            "Wk": np.asarray(Wk, np.float32),
            "bk": np.asarray(bk, np.float32),
            "Wv": np.asarray(Wv, np.float32),
            "bv": np.asarray(bv, np.float32),
        })
    res = run_bass_kernel_spmd(nc, in_maps, list(range(NCORES)))
    out = np.stack([res.results[b]["out"] for b in range(NCORES)], axis=0)
    return out.astype(np.float32)


# revision 16
# speedup vs baseline: 1.0607x; 1.0607x over previous
"""CrossAttentionHead TRN2 kernel.

Full inputs -> full output. Shards batch (B=8) across 8 NeuronCores,
one batch element per core (pure data parallel, no collectives).

Layout choice: each core's x shard is staged host-side as xT = x.T
([E, S], part of sharding prep), so the kernel streams it straight into
the e-on-partitions layout every matmul needs -- no on-chip transpose
pass over x.

Per-core algorithm (xT: [E=768, S=2048], W*: [E, H=128]):
  qT   = Wq.T @ xT + bq                    ([H, S], weights stationary)
  kT   = Wk.T @ xT + bk
  vT   = Wv.T @ xT + bv  -> vN = transpose(vT)   ([S, H] natural)
  for each sq block (512 wide):
    for each sk tile pair (2x128):
      sT   = kT_tile.T @ qT_block          (scores TRANSPOSED [sk, sq])
      es   = exp(sT / sqrt(E))             (ScalarE, scale fused, 1024 wide)
      acc += es                            (DVE, for row sums)
      oT  += vN_tile.T @ es                (PV accumulate, [H, sq])
    rowsum = ones.T @ acc                  ([1, sq] via PE, ones stationary)
    rsT    = transpose(rowsum)             (PE, [sq,1] tiles)
    out    = transpose(oT) * (1/rsT)       -> DMA

Matmul inputs use float32r (fp32 bits streamed through the PE in one
pass, ~2 cyc/row measured, vs plain fp32's 2 half-speed passes at
4 cyc/row; ~1.5e-4 relative rounding per matmul).
Softmax skips max-subtraction: energy/sqrt(768) ~ N(0, 0.41^2) so exp
is safely in range; matches jax.nn.softmax to fp32 rounding.
"""

import sys

if '/opt/trn_rl_repo' not in sys.path:
    sys.path.insert(0, '/opt/trn_rl_repo')

import numpy as np

B, S, E, H = 8, 2048, 768, 128
NCORES = 8
ST = S // 128          # 16 sequence tiles
EC = E // 128          # 6 embed chunks
QB = 4                 # sq blocks
QW = S // QB           # 512 sq block width
SCALE = float(1.0 / np.sqrt(np.float32(E)))

_CACHE = {}
F32R = True


def _build(f32r=F32R):
    import concourse.bacc as bacc
    import concourse.mybir as mybir
    import concourse.tile as tile
    from concourse.masks import make_identity

    dt = mybir.dt
    f32 = dt.float32
    fmm = dt.float32r if f32r else dt.float32
    AF = mybir.ActivationFunctionType

    nc = bacc.Bacc(None, target_bir_lowering=False)
    xT_d = nc.dram_tensor("xT", [E, S], f32, kind="ExternalInput")
    w_d = {}
    b_d = {}
    for nm in ("q", "k", "v"):
        w_d[nm] = nc.dram_tensor(f"W{nm}", [E, H], f32, kind="ExternalInput")
        b_d[nm] = nc.dram_tensor(f"b{nm}", [H], f32, kind="ExternalInput")
    out_d = nc.dram_tensor("out", [S, H], f32, kind="ExternalOutput")

    with tile.TileContext(nc) as tc:
        with tc.tile_pool(name="const", bufs=1) as constp, \
             tc.tile_pool(name="big", bufs=1) as bigp:
            ident = constp.tile([128, 128], f32)
            make_identity(nc, ident[:])
            ones = constp.tile([128, 1], f32)
            nc.vector.memset(ones[:], 1.0)

            # HAM warm-up: dense junk matmuls flip the PE clock gate to
            # 8/8 (~3.4us of activity) while the input DMAs stream in.
            with tc.tile_pool(name="warm_ps", bufs=1, space="PSUM") as wmp:
                wps = wmp.tile([128, 128], f32, tag="warm")
                for _ in range(17):
                    nc.tensor.matmul(wps[:], ident[:], ident[:],
                                     start=True, stop=True)
                wsb = constp.tile([128, 128], f32, name="warm_sink")
                nc.vector.tensor_copy(wsb[:], wps[:])

            w_mm = {}
            b_sb = {}
            for nm in ("q", "k", "v"):
                w_mm[nm] = constp.tile([128, EC, H], fmm, name=f"w_{nm}")
                nc.sync.dma_start(
                    out=w_mm[nm][:],
                    in_=w_d[nm].rearrange("(c p) d -> p c d", p=128).bitcast(fmm))

            xT = []
            for c in range(EC):
                t = bigp.tile([128, S], fmm, name=f"xT{c}")
                for n in range(4):
                    nc.sync.dma_start(
                        out=t[:, n * 512:(n + 1) * 512],
                        in_=xT_d[c * 128:(c + 1) * 128,
                                 n * 512:(n + 1) * 512].bitcast(fmm))
                xT.append(t)

            for nm in ("q", "k", "v"):
                b_sb[nm] = constp.tile([128, 1], f32, name=f"b_{nm}")
                nc.sync.dma_start(out=b_sb[nm][:], in_=b_d[nm][:, None])

            # Projections, split per 512-wide n block: qT/kT/vT = W.T@xT + b
            qT = [bigp.tile([128, QW], fmm, name=f"qT{n}") for n in range(4)]
            kT = [bigp.tile([128, QW], fmm, name=f"kT{n}") for n in range(4)]
            vT = [bigp.tile([128, QW], f32, name=f"vT{n}") for n in range(4)]
            # q/k first with chunk-outer accumulation: every psum tile
            # advances as each xT chunk's DMA lands (no stall on chunk 5)
            with tc.tile_pool(name="proj_ps", bufs=1, space="PSUM") as projp:
                ps_qk = {(nm, n): projp.tile([128, QW], f32,
                                             name=f"ps_{nm}{n}", tag=f"p{nm}{n}")
                         for nm in ("q", "k") for n in range(4)}
                for c in range(EC):
                    for nm in ("q", "k"):
                        for n in range(4):
                            nc.tensor.matmul(
                                ps_qk[(nm, n)][:], w_mm[nm][:, c, :],
                                xT[c][:, n * 512:(n + 1) * 512],
                                start=(c == 0), stop=(c == EC - 1))
                for nm, dst in (("q", qT), ("k", kT)):
                    for n in range(4):
                        nc.vector.tensor_scalar_add(
                            dst[n][:], ps_qk[(nm, n)][:], b_sb[nm][:])
                for n in range(4):
                    ps = projp.tile([128, QW], f32, name=f"ps_v{n}",
                                    tag=f"pq{n}")
                    for c in range(EC):
                        nc.tensor.matmul(
                            ps[:], w_mm["v"][:, c, :],
                            xT[c][:, n * 512:(n + 1) * 512],
                            start=(c == 0), stop=(c == EC - 1))
                    nc.scalar.activation(
                        vT[n][:], ps[:], AF.Identity,
                        bias=b_sb["v"][:], scale=1.0)

            # v natural [S, H], one tile per sk tile
            vN = [bigp.tile([128, H], fmm, name=f"vN{t}") for t in range(ST)]
            with tc.tile_pool(name="vt_ps", bufs=4, space="PSUM") as vtp:
                for t in range(ST):
                    pt = vtp.tile([128, 128], f32, tag="vt")
                    nc.tensor.transpose(
                        pt[:], vT[t // 4][:, (t % 4) * 128:(t % 4 + 1) * 128],
                        ident[:])
                    nc.vector.tensor_copy(vN[t][:], pt[:])

            # Main attention loop; kt pairs share one 1024-wide psum tile
            # so exp runs at 1024 elems/op
            with tc.tile_pool(name="s_ps", bufs=2, space="PSUM") as sp, \
                 tc.tile_pool(name="o_ps", bufs=2, space="PSUM") as op, \
                 tc.tile_pool(name="f_ps", bufs=2, space="PSUM") as fp, \
                 tc.tile_pool(name="es_sb", bufs=4) as esp, \
                 tc.tile_pool(name="acc_sb", bufs=2) as accp, \
                 tc.tile_pool(name="o_sb", bufs=2) as osp, \
                 tc.tile_pool(name="small", bufs=4) as smp, \
                 tc.tile_pool(name="fin", bufs=4) as finp:
                for qb in range(QB):
                    oT_ps = op.tile([128, QW], f32, tag="opv")
                    acc2 = accp.tile([128, 2 * QW], f32, tag="acc")
                    for kp in range(ST // 2):
                        kt0, kt1 = 2 * kp, 2 * kp + 1
                        s_ps = sp.tile([128, 2 * QW], f32, tag="s")
                        for i, kt in ((0, kt0), (1, kt1)):
                            nc.tensor.matmul(
                                s_ps[:, i * QW:(i + 1) * QW],
                                kT[kt // 4][:, (kt % 4) * 128:(kt % 4 + 1) * 128],
                                qT[qb][:], start=True, stop=True)
                        es = esp.tile([128, 2 * QW], fmm, tag="es")
                        nc.scalar.activation(es[:], s_ps[:], AF.Exp,
                                             scale=SCALE)
                        if kp == 0:
                            nc.vector.tensor_copy(acc2[:], es[:])
                        else:
                            nc.vector.tensor_add(acc2[:], acc2[:], es[:])
                        for i, kt in ((0, kt0), (1, kt1)):
                            nc.tensor.matmul(
                                oT_ps[:], vN[kt][:], es[:, i * QW:(i + 1) * QW],
                                start=(kt == 0), stop=(kt == ST - 1))
                    # row sums: ones stationary (1-column weight load),
                    # both acc halves accumulate into one [1, 512] bank
                    rs_ps = fp.tile([1, QW], f32, tag="fin")
                    nc.tensor.matmul(rs_ps[:], ones[:], acc2[:, :QW],
                                     start=True, stop=False)
                    nc.tensor.matmul(rs_ps[:], ones[:], acc2[:, QW:],
                                     start=False, stop=True)
                    rs_row = smp.tile([1, QW], f32, tag="rsrow")
                    nc.vector.tensor_copy(rs_row[:], rs_ps[:])
                    oT_sb = osp.tile([128, QW], f32, tag="ot")
                    nc.vector.tensor_copy(oT_sb[:], oT_ps[:])
                    for st in range(4):
                        rsT_ps = fp.tile([128, 1], f32, tag="fin")
                        nc.tensor.transpose(
                            rsT_ps[:], rs_row[:, st * 128:(st + 1) * 128],
                            ident[:1, :1])
                        rcpT = smp.tile([128, 1], f32, tag="rcp")
                        nc.vector.reciprocal(rcpT[:], rsT_ps[:])
                        ot_ps = fp.tile([128, 128], f32, tag="fin")
                        nc.tensor.transpose(
                            ot_ps[:], oT_sb[:, st * 128:(st + 1) * 128],
                            ident[:])
                        o_sb = finp.tile([128, 128], f32, tag="osb")
                        nc.vector.tensor_scalar_mul(o_sb[:], ot_ps[:], rcpT[:])
                        r0 = (qb * 4 + st) * 128
                        nc.sync.dma_start(
                            out=out_d[r0:r0 + 128, :], in_=o_sb[:])

    nc.finalize()
    return nc


def _get_nc():
    if "nc" not in _CACHE:
        _CACHE["nc"] = _build()
    return _CACHE["nc"]


def kernel(x, enc_output, Wq, bq, Wk, bk, Wv, bv):
    from concourse.bass_utils import run_bass_kernel_spmd

    nc = _get_nc()
    x = np.asarray(x, dtype=np.float32)
    in_maps = []
    for b in range(NCORES):
        in_maps.append({
            "xT": np.ascontiguousarray(x[b].T),
            "Wq": np.asarray(Wq, np.float32),
            "bq": np.asarray(bq, np.float32),
            "Wk": np.asarray(Wk, np.float32),
            "bk": np.asarray(bk, np.float32),
            "Wv": np.asarray(Wv, np.float32),
            "bv": np.asarray(bv, np.float32),
        })
    res = run_bass_kernel_spmd(nc, in_maps, list(range(NCORES)))
    out = np.stack([res.results[b]["out"] for b in range(NCORES)], axis=0)
    return out.astype(np.float32)
